# revision 23
# baseline (speedup 1.0000x reference)
"""Trainium2 Bass kernel for DihedralToCartesian (NeRF-style dihedral->xyz chain).

Full-input contract: kernel(angles[65536,252], prev_three[65536,3,3]) -> [65536,126,3].
Internally: batch is sharded 8 ways (8192 rows/core, pure data parallelism).

Math (validated vs the JAX reference in numpy, incl. fp16 rounding: rel ~4.5e-3):
normalize the dihedral (sin,cos) EXACTLY, which makes every per-step
normalizer of the affine-chain formulation identically 1; the recurrence is a
pure rotation chain.  With f3m := -f3 the (f2,f3) half is one 2x2 rotation:
    (h | f3m') = (ch*f2 + sh*f3m | -sh*f2 + ch*f3m)
    p'   = p + bond*cosA*f1 + bond*sinA*h
    f1'  = -cosA*f1 - sinA*h
    f2'  =  sinA*f1 - cosA*h

Perf structure (917us baseline -> 360us measured; DVE-saturated):
- fp16 state everywhere on the chain: TENSOR_TENSOR hits the DVE 2x_1p mode
  (fp32 TT is stuck at 1x).  Positions accumulate in fp16; the per-block DVE
  transpose-copy converts to fp32 for DMA.
- The (f2,f3m) 2x2 rotation is ONE fused [128,2,2,3,64] product against a
  4-slot scalar plane (-sh,ch,ch,sh) + one pair-sum that lands (f3m'|h)
  directly in the next state tile W = (f2|f3m|h).
- All planes the atom loop touches are atom-major so every broadcast operand
  has unit minor stride (strided src1 cost +160ns/op; ScalarE is ~5x worse).
- One ACT table set for the whole kernel: rsqrt = ACT Sqrt + DVE
  reciprocal_approx_fast (Ln/Exp picked different sets -> 41 table loads =
  52us; the ACT Rsqrt table itself is banned as inaccurate).
- No GPSIMD at all (it shares an SBUF port with the DVE: 54% Pool duty
  inflated DVE ops 2x in the baseline).  All DMA on the sync queue
  (ACT-issued DMA occupies the ACT sequencer and stalls the f1-scalings).
"""

import os
import sys

import numpy as np

for _p in ("/opt/trn_rl_repo", os.path.expanduser("~/.axon_site/_ro/trn_rl_repo")):
    if os.path.isdir(_p) and _p not in sys.path:
        sys.path.insert(0, _p)

import concourse.bass as bass
import concourse.bacc as bacc
import concourse.mybir as mybir
import concourse.tile as tile
from concourse.bass_utils import run_bass_kernel_spmd

F32 = mybir.dt.float32
F16 = mybir.dt.float16
AOP = mybir.AluOpType
AF = mybir.ActivationFunctionType

N_CORES = 8
B_FULL = 65536
BS = B_FULL // N_CORES  # 8192 rows per core
N = 126                 # atoms
P = 128                 # partitions
J = BS // P             # 64 batch columns per partition
BLK = 18                # atoms per output staging block
CH = 6                  # atoms per precompute chunk
NCH = N // CH           # 21 chunks

_ALPHA = np.array([2.028, 2.124, 1.941], np.float32)
_BOND = np.array([1.329, 1.458, 1.523], np.float32)
_CA = np.cos(_ALPHA)
_SA = np.sin(_ALPHA)
_BCA = _BOND * _CA
_BSA = _BOND * _SA


def _emit(nc: bass.Bass):
    angles = nc.dram_tensor("angles", [BS, 2 * N], F32, kind="ExternalInput").ap()
    prev = nc.dram_tensor("prev_three", [BS, 3, 3], F32, kind="ExternalInput").ap()
    out = nc.dram_tensor("out", [BS, N, 3], F32, kind="ExternalOutput").ap()

    ang_r = angles.rearrange("(p j) c -> p j c", p=P)          # [128, 64, 252]
    prev_r = prev.rearrange("(p j) r c -> p j (r c)", p=P)     # [128, 64, 9]
    out_r = out.rearrange("(p j) a c -> p j (a c)", p=P)       # [128, 64, 378]

    with tile.TileContext(nc) as tc:
        with (
            tc.tile_pool(name="planes", bufs=1) as planes,
            tc.tile_pool(name="stag", bufs=2) as stagp,
            tc.tile_pool(name="chunk", bufs=2) as chunk,
            tc.tile_pool(name="state", bufs=2) as state,
            tc.tile_pool(name="scratch", bufs=2) as scratch,
        ):
            # persistent planes.  raw angles f = 126*j + a (DMA-friendly);
            # scalar plane cs is atom-major with 4 slots per atom:
            # (-sh, ch, ch, sh) feeding the fused 2x2-rotation product.
            rawS = planes.tile([P, J * N], F32, tag="rawS")
            rawC = planes.tile([P, J * N], F32, tag="rawC")
            cs = planes.tile([P, N * 4 * J], F16, tag="cs")
            pv = planes.tile([P, J * 9], F32, tag="pv")

            nc.sync.dma_start(
                out=pv[:].rearrange("p (j x) -> p j x", x=9), in_=prev_r
            )
            rs_v = rawS[:].rearrange("p (j a) -> p j a", a=N)
            rc_v = rawC[:].rearrange("p (j a) -> p j a", a=N)
            for lo, hi in ((0, 12), (12, 24), (24, 48), (48, 90), (90, N)):
                nc.sync.dma_start(out=rs_v[:, :, lo:hi], in_=ang_r[:, :, lo:hi])
                nc.sync.dma_start(
                    out=rc_v[:, :, lo:hi], in_=ang_r[:, :, N + lo : N + hi]
                )

            cs_r = cs[:].rearrange("p (a s j) -> p a s j", s=4, j=J)  # [P,N,4,J]

            def cs_slot(g, s):  # [P, CH, J] view of slot s for chunk g
                return cs_r[:, CH * g : CH * (g + 1), s, :]

            def raw_chunk_T(t, g):  # [P, CH, J] transposed view of a raw chunk
                return t[:].rearrange("p (j a) -> p a j", a=N)[
                    :, CH * g : CH * (g + 1), :
                ]

            # ---- chunk precompute (per chunk g), everything atom-major:
            # DVE: sT/cT = transposed fp16 copies of the raw chunks
            # ACT: ss = sT^2; cc = cT^2 (fp32)     [Square]
            # DVE: d = ss + cc (fp32)
            # ACT: sq = sqrt(d)                    [Sqrt -- same table set]
            # DVE: rv = recip_approx(sq) fp32; rv16 = copy fp16
            # DVE: cs.slot1 = cT*rv16; cs.slot0 = sT*rv16  (fp16 2x)
            # DVE: cs.slot2 = -slot0 (tensor_scalar)
            t16_tiles, ss_tiles, d_tiles, rv_tiles = {}, {}, {}, {}

            def p1_sT(g):
                sT = chunk.tile([P, CH, J], F16, tag="sT", name=f"sT{g}")
                nc.vector.tensor_copy(sT[:], raw_chunk_T(rawS, g))
                cT = chunk.tile([P, CH, J], F16, tag="cT", name=f"cT{g}")
                nc.vector.tensor_copy(cT[:], raw_chunk_T(rawC, g))
                t16_tiles[g] = (sT, cT)

            def p1_sq(g):  # ACT squares (contiguous fp16 in, fp32 out)
                sT, cT = t16_tiles[g]
                ss = chunk.tile([P, CH, J], F32, tag="ss", name=f"ss{g}")
                nc.scalar.square(ss[:], sT[:])
                cc = chunk.tile([P, CH, J], F32, tag="cc", name=f"cc{g}")
                nc.scalar.square(cc[:], cT[:])
                ss_tiles[g] = (ss, cc)

            def p1_d(g):
                ss, cc = ss_tiles.pop(g)
                d = chunk.tile([P, CH, J], F32, tag="d", name=f"d{g}")
                nc.vector.tensor_add(d[:], ss[:], cc[:])
                d_tiles[g] = d

            def p1_sqrt(g):  # ACT
                d = d_tiles.pop(g)
                sq = chunk.tile([P, CH, J], F32, tag="ss", name=f"sq{g}")
                nc.scalar.sqrt(sq[:], d[:])
                d_tiles[(g, "sq")] = sq

            def p1_rv(g):  # DVE: 1/sqrt(d) fp32 then fp16
                sq = d_tiles.pop((g, "sq"))
                rv = chunk.tile([P, CH, J], F32, tag="cc", name=f"rv32_{g}")
                nc.vector.reciprocal_approx_fast(rv[:], sq[:])
                rv16 = chunk.tile([P, CH, J], F16, tag="rv16", name=f"rv16_{g}")
                nc.vector.tensor_copy(rv16[:], rv[:])
                rv_tiles[g] = rv16

            def p2_mulc1(g):  # ch -> slot1
                sT, cT = t16_tiles[g]
                nc.vector.tensor_mul(cs_slot(g, 1), cT[:], rv_tiles[g][:])

            def p2_mulc2(g):  # ch -> slot2
                sT, cT = t16_tiles[g]
                nc.vector.tensor_mul(cs_slot(g, 2), cT[:], rv_tiles[g][:])

            def p2_muls(g):  # sh -> slot3
                sT, cT = t16_tiles.pop(g)
                nc.vector.tensor_mul(cs_slot(g, 3), sT[:], rv_tiles.pop(g)[:])

            def p2_neg(g):  # -sh -> slot0
                nc.vector.tensor_scalar_mul(cs_slot(g, 0), cs_slot(g, 3), -1.0)

            def p1_all(g):
                p1_sT(g); p1_sq(g); p1_d(g); p1_sqrt(g); p1_rv(g)

            # bootstrap chunks 0 and 1 fully
            for g in (0, 1):
                p1_all(g); p2_mulc1(g); p2_mulc2(g); p2_muls(g); p2_neg(g)

            # ---- initial frame from prev_three (fp32 scratch) ----------------
            pv_r = pv[:].rearrange("p (j x) -> p x j", x=9)      # [128, 9, 64]
            a_ap = pv_r[:, 0:3, :]
            b_ap = pv_r[:, 3:6, :]
            c_ap = pv_r[:, 6:9, :]

            def cross(dst, x, y, eps):
                for c in range(3):
                    c1, c2_ = (c + 1) % 3, (c + 2) % 3
                    m = scratch.tile([P, 1, J], F32, tag="cr_m")
                    qt = scratch.tile([P, 1, J], F32, tag="cr_q")
                    nc.vector.tensor_mul(m[:], x[:, c1 : c1 + 1, :], y[:, c2_ : c2_ + 1, :])
                    nc.vector.tensor_mul(qt[:], x[:, c2_ : c2_ + 1, :], y[:, c1 : c1 + 1, :])
                    nc.vector.scalar_tensor_tensor(
                        dst[:, c : c + 1, :], m[:], eps, qt[:], AOP.add, AOP.subtract
                    )

            def rsqrt3(dst, src3):
                sq = scratch.tile([P, 3, J], F32, tag="in_sq")
                nc.vector.tensor_mul(sq[:], src3[:], src3[:])
                s1 = scratch.tile([P, J], F32, tag="in_s1")
                nc.vector.tensor_add(s1[:], sq[:, 0, :], sq[:, 1, :])
                s2_ = scratch.tile([P, J], F32, tag="in_s2")
                nc.vector.tensor_add(s2_[:], s1[:], sq[:, 2, :])
                sqr = scratch.tile([P, J], F32, tag="in_lg")
                nc.scalar.sqrt(sqr[:], s2_[:])
                nc.vector.reciprocal_approx_fast(dst[:], sqr[:])

            vv = scratch.tile([P, 3, J], F32, tag="in_v")
            nc.vector.scalar_tensor_tensor(
                vv[:], b_ap, 1e-8, c_ap, AOP.add, AOP.subtract
            )
            rv1 = scratch.tile([P, J], F32, tag="in_rv")
            rsqrt3(rv1, vv)
            f1w = scratch.tile([P, 3, J], F32, tag="in_f1w")
            nc.vector.tensor_mul(
                f1w[:], vv[:], rv1[:].unsqueeze(1).broadcast_to([P, 3, J])
            )
            uu = scratch.tile([P, 3, J], F32, tag="in_u")
            nc.vector.tensor_sub(uu[:], b_ap, a_ap)
            ww = scratch.tile([P, 3, J], F32, tag="in_w")
            cross(ww, uu, f1w, 1e-8)
            rw = scratch.tile([P, J], F32, tag="in_rw")
            rsqrt3(rw, ww)
            f3w = scratch.tile([P, 3, J], F32, tag="in_f3w")
            nc.vector.tensor_mul(
                f3w[:], ww[:], rw[:].unsqueeze(1).broadcast_to([P, 3, J])
            )
            f2w = scratch.tile([P, 3, J], F32, tag="in_f2w")
            cross(f2w, f3w, f1w, 0.0)

            # fp16 state: f1 and W = (f2 | f3m | h-scratch), p0
            f1 = state.tile([P, 3, J], F16, tag="f1")
            nc.scalar.copy(f1[:], f1w[:])
            W = state.tile([P, 3, 3, J], F16, tag="W")
            nc.scalar.copy(W[:][:, 0, :, :], f2w[:])
            nc.scalar.mul(W[:][:, 1, :, :], f3w[:], -1.0)
            p0 = state.tile([P, 3, J], F16, tag="p0")
            nc.scalar.copy(p0[:], c_ap)

            # ---- main chain --------------------------------------------------
            p_prev_ap = p0[:]
            pos_tiles = [None, None]
            stag_tiles = [None, None]

            for i in range(N):
                g, gph = i // CH, i % CH
                k3 = i % 3
                ca, sa = float(_CA[k3]), float(_SA[k3])
                bca, bsa = float(_BCA[k3]), float(_BSA[k3])
                blk, al = i // BLK, i % BLK
                last = i == N - 1
                if al == 0:
                    pos_tiles[blk % 2] = stagp.tile(
                        [P, BLK, 3, J], F16, tag="pos", name=f"pos{blk}"
                    )
                pos = pos_tiles[blk % 2]

                cs4 = (
                    cs_r[:, i, :, :]
                    .rearrange("p (h d) j -> p h d j", h=2)
                    .unsqueeze(3)
                    .broadcast_to([P, 2, 2, 3, J])
                )

                # ACT: f1 scalings (dep: f1 from i-1, ready) + chunk squares
                if not last:
                    fc = scratch.tile([P, 3, J], F16, tag="fc")
                    nc.scalar.mul(fc[:], f1[:], ca)
                    t9s = scratch.tile([P, 3, J], F16, tag="t9s")
                    nc.scalar.mul(t9s[:], f1[:], sa)
                ft = scratch.tile([P, 3, J], F16, tag="ft")
                nc.scalar.mul(ft[:], f1[:], bca)
                if gph == 1 and g + 2 < NCH:
                    p1_sq(g + 2)
                if gph == 3 and g + 2 < NCH:
                    p1_sqrt(g + 2)

                # DVE: fused (f2,f3m) rotation.  out12[h,d] = W[h]*cs[2h+d],
                # pair-sum over h gives (f3m' | h) straight into Wn[1:3].
                out12 = scratch.tile([P, 2, 2, 3, J], F16, tag="out12")
                nc.vector.tensor_mul(
                    out12[:],
                    W[:][:, 0:2, :, :].unsqueeze(2).broadcast_to([P, 2, 2, 3, J]),
                    cs4,
                )
                Wn = state.tile([P, 3, 3, J], F16, tag="W")
                nc.vector.tensor_add(
                    Wn[:][:, 1:3, :, :], out12[:][:, 0, :, :, :], out12[:][:, 1, :, :, :]
                )
                h_ap = Wn[:][:, 2, :, :]

                tmp = scratch.tile([P, 3, J], F16, tag="tmp")
                nc.vector.tensor_add(tmp[:], ft[:], p_prev_ap)
                pn_ap = pos[:][:, al, :, :]
                nc.vector.scalar_tensor_tensor(
                    pn_ap, h_ap, bsa, tmp[:], AOP.mult, AOP.add
                )

                if not last:
                    f1n = state.tile([P, 3, J], F16, tag="f1")
                    nc.vector.scalar_tensor_tensor(
                        f1n[:], h_ap, -sa, fc[:], AOP.mult, AOP.subtract
                    )
                    nc.vector.scalar_tensor_tensor(
                        Wn[:][:, 0, :, :], h_ap, -ca, t9s[:], AOP.mult, AOP.add
                    )
                    f1 = f1n
                W = Wn

                # DVE side of the chunk pipeline: P1(g+2) and P2(g+1)
                if gph == 0 and g + 2 < NCH:
                    p1_sT(g + 2)
                if gph == 2 and g + 2 < NCH:
                    p1_d(g + 2)
                if gph == 4 and g + 2 < NCH:
                    p1_rv(g + 2)
                if gph == 1 and 2 <= g + 1 < NCH:
                    p2_mulc1(g + 1)
                if gph == 2 and 2 <= g + 1 < NCH:
                    p2_mulc2(g + 1)
                if gph == 3 and 2 <= g + 1 < NCH:
                    p2_muls(g + 1)
                if gph == 4 and 2 <= g + 1 < NCH:
                    p2_neg(g + 1)

                p_prev_ap = pn_ap

                if al == BLK - 1:
                    # DVE transpose (al,c,j)->(j,al,c) + fp16->fp32, then DMA.
                    # j is split (2,32) so the most-major dim is size 2
                    # (2x_2p eligibility); DMA is split by row-halves across
                    # the sync and gpsimd queues so the tail DMA halves.
                    stag_tiles[blk % 2] = stagp.tile(
                        [P, J * 3 * BLK], F32, tag="stag", name=f"stag{blk}"
                    )
                    stag = stag_tiles[blk % 2]
                    pos_t = pos[:].rearrange(
                        "p al c (jh jl) -> p jh jl al c", jh=2
                    )
                    stag_t = stag[:].rearrange(
                        "p (jh jl al c) -> p jh jl al c", jh=2, al=BLK, c=3
                    )
                    nc.vector.tensor_copy(stag_t, pos_t)
                    stag_v = stag[:].rearrange("p (j x) -> p j x", x=3 * BLK)
                    xsl = slice(3 * BLK * blk, 3 * BLK * (blk + 1))
                    nc.sync.dma_start(
                        out=out_r[:, 0 : J // 2, xsl],
                        in_=stag_v[:, 0 : J // 2, :],
                    )
                    nc.gpsimd.dma_start(
                        out=out_r[:, J // 2 :, xsl],
                        in_=stag_v[:, J // 2 :, :],
                    )
    return nc


_NC_CACHE: dict = {}


def _get_nc():
    if "nc" not in _NC_CACHE:
        nc = bacc.Bacc("TRN2", target_bir_lowering=False, debug=False)
        _emit(nc)
        nc.compile()
        _NC_CACHE["nc"] = nc
    return _NC_CACHE["nc"]


def run_sharded(angles: np.ndarray, prev_three: np.ndarray, **kw):
    """Shard inputs over 8 cores, run, return BassKernelResults."""
    angles = np.ascontiguousarray(angles, np.float32)
    prev_three = np.ascontiguousarray(prev_three, np.float32)
    assert angles.shape == (B_FULL, 2 * N) and prev_three.shape == (B_FULL, 3, 3)
    in_maps = [
        {
            "angles": angles[i * BS : (i + 1) * BS],
            "prev_three": prev_three[i * BS : (i + 1) * BS],
        }
        for i in range(N_CORES)
    ]
    return run_bass_kernel_spmd(_get_nc(), in_maps, core_ids=list(range(N_CORES)), **kw)


def kernel(angles: np.ndarray, prev_three: np.ndarray) -> np.ndarray:
    res = run_sharded(angles, prev_three)
    return np.concatenate([r["out"] for r in res.results], axis=0)
